# revision 2
# baseline (speedup 1.0000x reference)
"""VQ codebook lookup: fp8 DoubleRow GEMM + 2-candidate exact rescore.

Per 128-token tile:
  1. PE: approx scores s = fp8(x)*fp8(2c) - csq for 8192 codes, via DoubleRow
     fp8 matmuls (256-dim contraction/instr).  csq enters as a 3-term fp8
     split h+m+l scaled by a 4.0 selector row (exact to ~0.125).
  2. ACT drains PSUM -> SBUF fp16 scores.
  3. DVE: segmented max (16-wide) -> segmax[P,512]; max8 gives the top-8
     segment maxima v0..v7.  tau = v0 - DELTA.
  4. Two stt threshold passes with packed iota decode up to TWO in-band
     candidate indices: S counts/sums over [tau,inf), S2 over
     [max((v0+v1)/2, tau), inf) isolates the top-1.
  5. GPSIMD gathers augmented rows [c, -csq, 1] for both candidates; DVE
     rescores exactly in f32 (dot with [2x, 1, SHIFT]); winner's codebook row
     is gathered and stored.
  6. Tokens with >=3 in-band candidates / ties / tiny exact gap are flagged;
     host rescues them with an exact f64 dgemm (~10% of tokens).

Error budget: fp8 dot err sigma~1.15 (6sigma=6.9), csq split 0.125, fp16
drain 0.25, compare slack 0.5 => E=7.8; DELTA=2E~=15.6 -> use 16.
"""

import numpy as np

import concourse.bacc as bacc
import concourse.bass as bass
import concourse.mybir as mybir
from concourse.tile import TileContext

P = 128
D = 512
K = 8192
N_TOKENS = 32768
N_CORES = 8
T_PER_CORE = N_TOKENS // N_CORES
N_TILES_FULL = T_PER_CORE // P      # 32
QUARTER = 2048
N_Q = K // QUARTER                  # 4
C2 = 2                              # 256-dim chunk pairs
SB = 512                            # PSUM bank block
N_SB = QUARTER // SB                # 4
SEGW = 16
NSEG = K // SEGW                    # 512
AUGW = 516                          # [c(512), -csq, 1, pad2]

F32 = mybir.dt.float32
FP16 = mybir.dt.float16
FP8 = mybir.dt.float8e4
U32 = mybir.dt.uint32
DR = mybir.MatmulPerfMode.DoubleRow
AX = mybir.AxisListType.X
ALU = mybir.AluOpType

DELTA = 11.0                        # candidate band below fp8 max
GAPEPS = 0.05                       # exact-rescore ambiguity margin
SHIFT = 1000.0                      # makes exact scores positive
BPACK = float(1 << 17)


def build_bass(n_tiles=N_TILES_FULL, repeat=1):
    nc = bacc.Bacc()
    x8_tiles = nc.declare_dram_parameter(
        "x8_tiles", [n_tiles, P, C2, 2, P], FP8, isOutput=False)
    xf_tiles = nc.declare_dram_parameter(
        "xf_tiles", [n_tiles, P, AUGW], F32, isOutput=False)
    cb8 = nc.declare_dram_parameter(
        "cb8", [C2, N_Q, P, 2, QUARTER], FP8, isOutput=False)
    csq8 = nc.declare_dram_parameter(
        "csq8", [N_Q, P, 2, QUARTER], FP8, isOutput=False)
    sel4 = nc.declare_dram_parameter("sel4", [P, 2, P], FP8, isOutput=False)
    iota_b = nc.declare_dram_parameter("iota_b", [P, N_Q, QUARTER], F32,
                                       isOutput=False)
    cbaug = nc.declare_dram_parameter("cbaug", [K, AUGW], F32, isOutput=False)
    codebook = nc.declare_dram_parameter("codebook", [K, D], F32,
                                         isOutput=False)
    out = nc.declare_dram_parameter("out", [n_tiles * P, D], F32,
                                    isOutput=True)
    out_flags = nc.declare_dram_parameter(
        "out_flags", [P, n_tiles], F32, isOutput=True)

    with TileContext(nc) as tc:
        with (
            tc.tile_pool(name="const", bufs=1) as cpool,
            tc.tile_pool(name="xp", bufs=3) as xpool,
            tc.tile_pool(name="xfp", bufs=3) as xfpool,
            tc.tile_pool(name="sc", bufs=2) as scpool,
            tc.tile_pool(name="sm", bufs=3) as smpool,
            tc.tile_pool(name="ga", bufs=2) as gapool,
            tc.tile_pool(name="op", bufs=2) as opool,
            tc.tile_pool(name="dm", bufs=1) as dmpool,
            tc.tile_pool(name="ps", bufs=2, space="PSUM") as pspool,
        ):
            sel_sb = cpool.tile([P, 2, P], FP8, tag="sel4")
            nc.sync.dma_start(out=sel_sb, in_=sel4[:, :, :])
            iota_sb = cpool.tile([P, N_Q, QUARTER], F32, tag="iota")
            nc.scalar.dma_start(out=iota_sb, in_=iota_b[:, :, :])
            csq_sb = {}
            for q in range(N_Q):
                t = cpool.tile([P, 2, QUARTER], FP8, tag=f"csq_{q}")
                nc.scalar.dma_start(out=t, in_=csq8[q])
                csq_sb[q] = t
            cb_sb = {}
            dma_engs = [nc.sync, nc.scalar, nc.gpsimd]
            j = 0
            for q in range(N_Q):
                for c in range(C2):
                    t = cpool.tile([P, 2, QUARTER], FP8, tag=f"cb_{c}_{q}")
                    dma_engs[j % 3].dma_start(out=t, in_=cb8[c, q])
                    cb_sb[c, q] = t
                    j += 1
            flags_sb = cpool.tile([P, n_tiles], F32, tag="flags")
            dummy = dmpool.tile([P, 1], F32, tag="dummy")
            dummy5 = dmpool.tile([P, 1], F32, tag="dummy5")

            tts = [t for _ in range(repeat) for t in range(n_tiles)]
            for it, tt in enumerate(tts):
                xt = xpool.tile([P, C2, 2, P], FP8, tag="xt")
                nc.sync.dma_start(out=xt, in_=x8_tiles[tt])
                xf = xfpool.tile([P, AUGW], F32, tag="xf")
                nc.sync.dma_start(out=xf, in_=xf_tiles[tt])

                sc16 = scpool.tile([P, N_Q, QUARTER], FP16, tag="sc16")
                for q in range(N_Q):
                    ps = pspool.tile([P, QUARTER], F32, tag="ps")
                    for c in range(C2):
                        for s in range(N_SB):
                            nc.tensor.matmul(
                                out=ps[:, s * SB:(s + 1) * SB],
                                lhsT=xt[:, c],
                                rhs=cb_sb[c, q][:, :, s * SB:(s + 1) * SB],
                                perf_mode=DR,
                                start=(c == 0),
                                stop=False,
                            )
                    for s in range(N_SB):
                        nc.tensor.matmul(
                            out=ps[:, s * SB:(s + 1) * SB],
                            lhsT=sel_sb,
                            rhs=csq_sb[q][:, :, s * SB:(s + 1) * SB],
                            perf_mode=DR,
                            start=False,
                            stop=True,
                        )
                    nc.scalar.copy(sc16[:, q], ps)

                # global max -> tau band
                qmax = smpool.tile([P, N_Q], F32, tag="qmax")
                nc.vector.tensor_reduce(out=qmax, in_=sc16, axis=AX,
                                        op=ALU.max)
                v0 = smpool.tile([P, 1], F32, tag="v0")
                nc.vector.tensor_reduce(out=v0, in_=qmax, axis=AX,
                                        op=ALU.max)
                tau = smpool.tile([P, 1], F32, tag="tau")
                nc.scalar.activation(
                    out=tau, in_=v0,
                    func=mybir.ActivationFunctionType.Copy, bias=-DELTA)

                # one packed threshold pass; decode one candidate per quarter
                sacc = smpool.tile([P, N_Q], F32, tag="sacc")
                for q in range(N_Q):
                    nc.vector.scalar_tensor_tensor(
                        out=dummy.broadcast_to((P, QUARTER)),
                        in0=sc16[:, q], scalar=tau, in1=iota_sb[:, q],
                        op0=ALU.is_ge, op1=ALU.mult,
                        accum_out=sacc[:, q:q + 1])

                cnt2 = smpool.tile([P, N_Q], F32, tag="cnt2")
                nc.vector.tensor_scalar_mul(cnt2, sacc, 1.0 / BPACK)
                cnt2u = smpool.tile([P, N_Q], U32, tag="cnt2u")
                nc.vector.tensor_copy(cnt2u, cnt2)
                cntf = smpool.tile([P, N_Q], F32, tag="cntf")
                nc.vector.tensor_copy(cntf, cnt2u)
                kq = smpool.tile([P, N_Q], F32, tag="kq")
                nc.vector.scalar_tensor_tensor(
                    out=kq, in0=cntf, scalar=-BPACK, in1=sacc,
                    op0=ALU.mult, op1=ALU.add)
                idx2c = smpool.tile([P, N_Q], F32, tag="idx2c")
                nc.vector.tensor_scalar(
                    out=idx2c, in0=kq, scalar1=0.0, scalar2=float(K - 1),
                    op0=ALU.max, op1=ALU.min)
                idx2u = smpool.tile([P, N_Q], U32, tag="idx2u")
                nc.vector.tensor_copy(idx2u, idx2c)

                # ok1: every quarter decodable (count <= 1)
                cmax = smpool.tile([P, 1], F32, tag="cmax")
                nc.vector.reduce_max(out=cmax, in_=cntf, axis=AX)
                ok1 = smpool.tile([P, 1], F32, tag="ok1")
                nc.vector.tensor_scalar(
                    out=ok1, in0=cmax, scalar1=1.5, scalar2=None,
                    op0=ALU.is_lt)

                # gather augmented rows, exact rescore (one DMA per slot)
                ga = gapool.tile([P, N_Q, AUGW], F32, tag="ga")
                for r in range(N_Q):
                    nc.gpsimd.indirect_dma_start(
                        out=ga[:, r], out_offset=None,
                        in_=cbaug[:, :],
                        in_offset=bass.IndirectOffsetOnAxis(
                            ap=idx2u[:, r:r + 1], axis=0),
                    )
                ex = smpool.tile([P, N_Q], F32, tag="ex")
                for r in range(N_Q):
                    nc.vector.scalar_tensor_tensor(
                        out=dummy.broadcast_to((P, AUGW)),
                        in0=ga[:, r], scalar=1.0, in1=xf,
                        op0=ALU.mult, op1=ALU.mult,
                        accum_out=ex[:, r:r + 1])

                win = smpool.tile([P, 1], F32, tag="win")
                nc.vector.reduce_max(out=win, in_=ex, axis=AX)
                msk = smpool.tile([P, N_Q], F32, tag="msk")
                nc.vector.scalar_tensor_tensor(
                    out=msk, in0=ex, scalar=win, in1=ex,
                    op0=ALU.is_lt, op1=ALU.mult)
                sec = smpool.tile([P, 1], F32, tag="sec")
                nc.vector.reduce_max(out=sec, in_=msk, axis=AX)
                gap = smpool.tile([P, 1], F32, tag="gap")
                nc.vector.tensor_sub(gap, win, sec)
                ok3 = smpool.tile([P, 1], F32, tag="ok3")
                nc.vector.tensor_scalar(
                    out=ok3, in0=gap, scalar1=GAPEPS, scalar2=None,
                    op0=ALU.is_ge)

                idxf2 = smpool.tile([P, N_Q], F32, tag="idxf2")
                nc.vector.tensor_copy(idxf2, idx2u)
                idxw = smpool.tile([P, 1], F32, tag="idxw")
                nc.vector.scalar_tensor_tensor(
                    out=msk, in0=ex, scalar=win, in1=idxf2,
                    op0=ALU.is_ge, op1=ALU.mult, accum_out=idxw)

                nc.vector.tensor_tensor(
                    out=flags_sb[:, tt:tt + 1], in0=ok1, in1=ok3,
                    op=ALU.mult)

                idxwc = smpool.tile([P, 1], F32, tag="idxwc")
                nc.vector.tensor_scalar(
                    out=idxwc, in0=idxw, scalar1=0.0, scalar2=float(K - 1),
                    op0=ALU.max, op1=ALU.min)
                idxu = smpool.tile([P, 1], U32, tag="idxu")
                nc.vector.tensor_copy(idxu, idxwc)
                rows = opool.tile([P, D], F32, tag="rows")
                nc.gpsimd.indirect_dma_start(
                    out=rows, out_offset=None,
                    in_=codebook[:, :],
                    in_offset=bass.IndirectOffsetOnAxis(ap=idxu, axis=0),
                )
                nc.sync.dma_start(out=out[tt * P:(tt + 1) * P, :], in_=rows)

            nc.sync.dma_start(out=out_flags[:, :], in_=flags_sb)

    return nc


def _fp8(a):
    return np.asarray(a, dtype=np.float32).astype(mybir.dt.np(FP8))


def prep_core_inputs(x_core, shared, n_tiles):
    x = np.asarray(x_core, dtype=np.float32)
    # x8[t, p, c2, i, m] = x[t*128+m, 256*c2 + 128*i + p]
    x8 = _fp8(np.ascontiguousarray(
        x.reshape(n_tiles, P, C2, 2, P).transpose(0, 4, 2, 3, 1)))
    xf = np.zeros((n_tiles, P, AUGW), dtype=np.float32)
    xf[:, :, :D] = 2.0 * x.reshape(n_tiles, P, D)
    xf[:, :, D] = 1.0
    xf[:, :, D + 1] = SHIFT
    return {"x8_tiles": x8, "xf_tiles": np.ascontiguousarray(xf), **shared}


def prep_shared(codebook):
    cb = np.ascontiguousarray(np.asarray(codebook, dtype=np.float32))
    cb2 = 2.0 * cb
    # cb8[c2, q, p, i, j] = cb2[q*2048+j, 256*c2 + 128*i + p]
    cb8_np = _fp8(np.ascontiguousarray(
        cb2.reshape(N_Q, QUARTER, C2, 2, P).transpose(2, 0, 4, 3, 1)))
    csq = (cb.astype(np.float64) ** 2).sum(axis=1).astype(np.float32)
    tgt = (-csq / 4.0).reshape(N_Q, QUARTER)
    h = _fp8(tgt)
    m = _fp8(tgt - h.astype(np.float32))
    lo = _fp8(tgt - h.astype(np.float32) - m.astype(np.float32))
    csq8_np = np.zeros((N_Q, P, 2, QUARTER), dtype=mybir.dt.np(FP8))
    csq8_np[:, 0, 0, :] = h
    csq8_np[:, 0, 1, :] = m
    csq8_np[:, 1, 0, :] = lo
    sel4_np = np.zeros((P, 2, P), dtype=mybir.dt.np(FP8))
    sel4_np[0, 0, :] = 4.0
    sel4_np[0, 1, :] = 4.0
    sel4_np[1, 0, :] = 4.0
    iota_np = np.broadcast_to(
        (BPACK + np.arange(K, dtype=np.float32))[None, :],
        (P, K)).reshape(P, N_Q, QUARTER).copy()
    cbaug = np.zeros((K, AUGW), dtype=np.float32)
    cbaug[:, :D] = cb
    cbaug[:, D] = -csq
    cbaug[:, D + 1] = 1.0
    return {"cb8": cb8_np, "csq8": csq8_np, "sel4": sel4_np,
            "iota_b": iota_np, "cbaug": cbaug, "codebook": cb}


_NC_CACHE = {}


def _get_nc(key):
    if key not in _NC_CACHE:
        nc = build_bass(*key)
        nc.finalize()
        _NC_CACHE[key] = nc
    return _NC_CACHE[key]


def _host_rescue(out_full, flags_full, x, codebook):
    bad = np.flatnonzero(flags_full != 1.0)
    if len(bad) == 0:
        return out_full, 0
    xb = x[bad].astype(np.float64)
    cb64 = codebook.astype(np.float64)
    csq = (cb64 * cb64).sum(1)
    sc = 2.0 * (xb @ cb64.T) - csq[None, :]
    idx = sc.argmax(1)
    out_full[bad] = codebook[idx]
    return out_full, len(bad)


def kernel(x, codebook):
    from concourse.bass_utils import run_bass_kernel_spmd

    x = np.ascontiguousarray(np.asarray(x, dtype=np.float32))
    codebook = np.ascontiguousarray(np.asarray(codebook, dtype=np.float32))
    assert x.shape == (N_TOKENS, D) and codebook.shape == (K, D)

    nc = _get_nc((N_TILES_FULL, 1))
    shared = prep_shared(codebook)

    in_maps = []
    for core in range(N_CORES):
        x_core = x[core * T_PER_CORE:(core + 1) * T_PER_CORE]
        in_maps.append(prep_core_inputs(x_core, shared, N_TILES_FULL))

    res = run_bass_kernel_spmd(nc, in_maps, list(range(N_CORES)))
    out_full = np.concatenate(
        [res.results[i]["out"] for i in range(N_CORES)], axis=0)
    flags_full = np.concatenate(
        [np.asarray(res.results[i]["out_flags"]).T.reshape(-1)
         for i in range(N_CORES)])
    out_full, n_rescued = _host_rescue(out_full, flags_full, x, codebook)
    kernel.last_rescued = n_rescued
    return out_full
